# revision 3
# baseline (speedup 1.0000x reference)
"""V5: bf16 rewrite, restructured for PE occupancy.

vs V4:
- PSUM pools merged across phases (pools are bank-granular): qk-proj and
  y-proj share a 3-buf pool; v-proj and attention-out share a 2-buf pool.
  Total 3+2+2+1 = 8 banks with double/triple buffering everywhere.
- Phase 2/3 emitted per query-slice: all 4 (batch, head) attention chains
  of one qs interleave on the PE, and the output projection for the
  finished qs follows immediately, so the scheduler always has dense
  independent matmul work; the serial tail is one batch's last qs
  projection only.
- y-proj groups are [128, 512] (one PSUM bank, N=512 matmuls), copies
  alternate DVE/ACT, y DMA on the otherwise-idle SP queue.
"""

import math
from contextlib import ExitStack

import numpy as np
import ml_dtypes

import concourse.bass as bass
import concourse.tile as tile
from concourse import bacc, mybir
from concourse.bass_utils import run_bass_kernel_spmd

B, L, H, NH, HD = 2, 2048, 2048, 16, 128
ROPE_THETA = 10000.0
N_CORES = 8
NH_LOC = NH // N_CORES          # 2
QKV_LOC = 3 * NH_LOC * HD       # 768
D_LOC = NH_LOC * HD             # 256
BL = B * L
P = 128
KC = H // P                     # 16
BLK = 256
NBLK = BL // BLK                # 16
BLK_PER_B = NBLK // B           # 8
QS = 512
NQS = L // QS
KT = L // P
NBH = B * NH_LOC                # 4

F32 = mybir.dt.float32
BF16 = mybir.dt.bfloat16
EXP = mybir.ActivationFunctionType.Exp
NEG = -30000.0


def _build():
    nc = bacc.Bacc("TRN2", target_bir_lowering=False, debug=False,
                   num_devices=N_CORES)

    xT = nc.dram_tensor("xT", [H, BL], BF16, kind="ExternalInput").ap()
    wT = nc.dram_tensor("wT", [H, QKV_LOC], BF16, kind="ExternalInput").ap()
    woT = nc.dram_tensor("woT", [D_LOC, H], BF16, kind="ExternalInput").ap()
    cosT = nc.dram_tensor("cosT", [HD, L], F32, kind="ExternalInput").ap()
    sinTs = nc.dram_tensor("sinTs", [HD, L], F32, kind="ExternalInput").ap()
    tri = nc.dram_tensor("tri", [P, P], BF16, kind="ExternalInput").ap()
    ones_in = nc.dram_tensor("ones", [P, P], BF16, kind="ExternalInput").ap()
    y = nc.dram_tensor("y", [BL, H], BF16, kind="ExternalOutput").ap()

    with tile.TileContext(nc) as tc, ExitStack() as ctx:
        g = ctx.enter_context(tc.tile_pool(name="g", bufs=1))
        wt = g.tile([P, KC, QKV_LOC], BF16)
        cost = g.tile([P, L], F32)
        sints = g.tile([P, L], F32)
        k_all = g.tile([P, NBH, L], BF16)
        q_all = g.tile([P, NBH, L], BF16)
        v_all = g.tile([P, NBH, KT, HD], BF16)
        o_all = g.tile([P, B, NH_LOC, L], BF16)
        wo = g.tile([P, NH_LOC, H], BF16)
        trimask = g.tile([P, P], BF16)
        ones = g.tile([P, P], BF16)

        p1x = ctx.enter_context(tc.tile_pool(name="p1x", bufs=2))
        p1q = ctx.enter_context(tc.tile_pool(name="p1q", bufs=2))
        p1t = ctx.enter_context(tc.tile_pool(name="p1t", bufs=4))
        p2e = ctx.enter_context(tc.tile_pool(name="p2e", bufs=8))
        p2t = ctx.enter_context(tc.tile_pool(name="p2t", bufs=4))
        p3y = ctx.enter_context(tc.tile_pool(name="p3y", bufs=3))
        # PSUM: bank-granular. psA: phase-1 qk pairs + phase-3 y groups;
        # psVO: phase-1 v + phase-2 po; psS: score tiles; psD: denominators.
        psA = ctx.enter_context(tc.tile_pool(name="psA", bufs=3, space="PSUM"))
        psVO = ctx.enter_context(tc.tile_pool(name="psVO", bufs=2, space="PSUM"))
        psS = ctx.enter_context(tc.tile_pool(name="psS", bufs=3, space="PSUM"))

        # ------- weight / x(block0) loads, interleaved in kc order -------
        xb0 = p1x.tile([P, KC, BLK], BF16, name="xb")
        for c0_, c1_ in ((0, 2), (2, 4), (4, 8), (8, 12), (12, 16)):
            nc.sync.dma_start(
                wt[:, c0_:c1_, :],
                wT[c0_ * P:c1_ * P, :].rearrange("(n p) d -> p n d", p=P))
            nc.sync.dma_start(
                xb0[:, c0_:c1_, :],
                xT[c0_ * P:c1_ * P, 0:BLK]
                .rearrange("(n p) t -> p n t", p=P))
        xb1 = p1x.tile([P, KC, BLK], BF16, name="xb")
        nc.sync.dma_start(xb1[:], xT[:, BLK:2 * BLK]
                          .rearrange("(n p) t -> p n t", p=P))
        for ch in range(4):
            sl = slice(ch * 512, (ch + 1) * 512)
            nc.sync.dma_start(cost[:, sl], cosT[:, sl])
            nc.sync.dma_start(sints[:, sl], sinTs[:, sl])
        nc.sync.dma_start(trimask[:], tri[:])
        nc.sync.dma_start(ones[:], ones_in[:])
        for hh in range(NH_LOC):
            nc.sync.dma_start(wo[:, hh, :], woT[hh * P:(hh + 1) * P, :])

        # ---------------- phase 1: QKV projection + RoPE ----------------
        for blk in range(NBLK):
            b, lo = divmod(blk, BLK_PER_B)
            lo *= BLK
            col = blk * BLK
            if blk == 0:
                xb = xb0
            elif blk == 1:
                xb = xb1
            else:
                xb = p1x.tile([P, KC, BLK], BF16, name="xb")
                nc.sync.dma_start(
                    xb[:], xT[:, col:col + BLK]
                    .rearrange("(n p) t -> p n t", p=P))

            qc = p1q.tile([P, 4, BLK], BF16, name="qc")
            for pair in range(2):     # dt 0,1 = q heads; 2,3 = k heads
                psum = psA.tile([P, QS], F32, name="acc")
                for j in range(2):
                    dt_i = pair * 2 + j
                    for kc in range(KC):
                        nc.tensor.matmul(
                            psum[:, j * BLK:(j + 1) * BLK],
                            lhsT=wt[:, kc, dt_i * P:dt_i * P + P],
                            rhs=xb[:, kc, :],
                            start=(kc == 0), stop=(kc == KC - 1))
                    nc.scalar.copy(qc[:, dt_i, :],
                                   psum[:, j * BLK:(j + 1) * BLK])
            # batched rotate-half swap for all 4 dim-tiles
            qsw = p1q.tile([P, 4, BLK], BF16, name="qsw")
            nc.sync.dma_start(qsw[0:64, :, :], qc[64:128, :, :])
            nc.sync.dma_start(qsw[64:128, :, :], qc[0:64, :, :])

            for dt_i in range(4):
                qk, hh = divmod(dt_i, 2)
                bh = b * NH_LOC + hh
                t1 = p1t.tile([P, BLK], F32, name="t1")
                nc.vector.tensor_mul(t1[:], qc[:, dt_i, :],
                                     cost[:, lo:lo + BLK])
                t2 = p1t.tile([P, BLK], F32, name="t2")
                nc.vector.tensor_mul(t2[:], qsw[:, dt_i, :],
                                     sints[:, lo:lo + BLK])
                if qk == 0:
                    nc.vector.tensor_add(q_all[:, bh, lo:lo + BLK],
                                         t1[:], t2[:])
                else:
                    nc.vector.tensor_add(k_all[:, bh, lo:lo + BLK],
                                         t1[:], t2[:])

            psv = psVO.tile([P, QS], F32, name="vo")
            for tt in range(BLK // P):
                for kc in range(KC):
                    nc.tensor.matmul(
                        psv[:, tt * D_LOC:(tt + 1) * D_LOC],
                        lhsT=xb[:, kc, tt * P:(tt + 1) * P],
                        rhs=wt[:, kc, 2 * D_LOC:3 * D_LOC],
                        start=(kc == 0), stop=(kc == KC - 1))
                kt = lo // P + tt
                for hh in range(NH_LOC):
                    nc.scalar.copy(
                        v_all[:, b * NH_LOC + hh, kt, :],
                        psv[:, tt * D_LOC + hh * HD:tt * D_LOC + (hh + 1) * HD])

        # ------- phase 2+3: attention and y-projection, per query slice ----
        copy_flip = 0
        for qs_i in range(NQS):
            qs = qs_i * QS
            nkt = (qs + QS) // P
            for b in ((1, 0) if qs_i == NQS - 1 else (0, 1)):
                for hh in range(NH_LOC):
                    bh = b * NH_LOC + hh
                    po = psVO.tile([P, QS], F32, name="vo")
                    pd = psA.tile([P, QS], F32, name="acc")
                    for k_i in range(nkt):
                        d = k_i * P - qs
                        c0 = max(d, 0)
                        psc = psS.tile([P, QS], F32, name="psc")
                        nc.tensor.matmul(
                            psc[:, c0:QS],
                            lhsT=k_all[:, bh, k_i * P:(k_i + 1) * P],
                            rhs=q_all[:, bh, qs + c0:qs + QS],
                            start=True, stop=True)
                        et = p2e.tile([P, QS], BF16, name="et")
                        nc.scalar.activation(et[:, c0:QS], psc[:, c0:QS], EXP)
                        if d >= 0:
                            nc.vector.tensor_mul(et[:, d:d + P],
                                                 et[:, d:d + P], trimask[:])
                        nc.tensor.matmul(po[:, c0:QS],
                                         lhsT=v_all[:, bh, k_i, :],
                                         rhs=et[:, c0:QS], start=(k_i == 0),
                                         stop=(k_i == nkt - 1))
                        nc.tensor.matmul(pd[:, c0:QS], lhsT=ones[:],
                                         rhs=et[:, c0:QS], start=(k_i == 0),
                                         stop=(k_i == nkt - 1))
                    rec = p2t.tile([P, QS], F32, name="rec")
                    nc.vector.reciprocal(rec[:], pd[:])
                    nc.vector.tensor_mul(o_all[:, b, hh, qs:qs + QS],
                                         po[:], rec[:])
                # y-projection for this batch's freshly finished qs tokens
                for tt in range(qs // P, (qs + QS) // P):
                    ybig = p3y.tile([P, H], BF16, name="ybig")
                    for oc in range(H // QS):
                        py_ = psA.tile([P, QS], F32, name="acc")
                        for hh in range(NH_LOC):
                            nc.tensor.matmul(
                                py_[:],
                                lhsT=o_all[:, b, hh, tt * P:(tt + 1) * P],
                                rhs=wo[:, hh, oc * QS:(oc + 1) * QS],
                                start=(hh == 0), stop=(hh == NH_LOC - 1))
                        if copy_flip % 2 == 0:
                            nc.vector.tensor_copy(
                                ybig[:, oc * QS:(oc + 1) * QS], py_[:])
                        else:
                            nc.scalar.copy(
                                ybig[:, oc * QS:(oc + 1) * QS], py_[:])
                        copy_flip += 1
                    nc.sync.dma_start(
                        y[b * L + tt * P: b * L + (tt + 1) * P, :], ybig[:])

    nc.compile()
    return nc


_NC = None


def _get_nc():
    global _NC
    if _NC is None:
        _NC = _build()
    return _NC


def _host_inputs(x, Wqkv, Wo):
    x = np.asarray(x, dtype=np.float32)
    Wqkv = np.asarray(Wqkv, dtype=np.float32)
    Wo = np.asarray(Wo, dtype=np.float32)

    xT = np.ascontiguousarray(x.reshape(BL, H).T.astype(ml_dtypes.bfloat16))

    inv_freq = 1.0 / (ROPE_THETA ** (np.arange(0, HD, 2, dtype=np.float32)
                                     / HD))
    t = np.arange(L, dtype=np.float32)
    freqs = np.outer(t, inv_freq).astype(np.float32)
    emb = np.concatenate([freqs, freqs], axis=-1)
    cosT = np.ascontiguousarray(np.cos(emb).T.astype(np.float32))
    sinT = np.sin(emb).T.astype(np.float32)
    sinTs = np.ascontiguousarray(np.concatenate([-sinT[:64], sinT[64:]], 0))

    kk = np.arange(P)[:, None]
    qq = np.arange(P)[None, :]
    tri = np.where(qq >= kk, 1.0, 0.0).astype(ml_dtypes.bfloat16)

    scale = np.float32(1.0 / math.sqrt(HD))
    in_maps = []
    for c in range(N_CORES):
        r0 = c * D_LOC
        wq = Wqkv[r0:r0 + D_LOC] * scale
        wk = Wqkv[H + r0:H + r0 + D_LOC]
        wv = Wqkv[2 * H + r0:2 * H + r0 + D_LOC]
        wT_c = np.ascontiguousarray(
            np.concatenate([wq, wk, wv], 0).T.astype(ml_dtypes.bfloat16))
        woT_c = np.ascontiguousarray(
            Wo[:, r0:r0 + D_LOC].T.astype(ml_dtypes.bfloat16))
        in_maps.append({
            "xT": xT, "wT": wT_c, "woT": woT_c,
            "cosT": cosT, "sinTs": sinTs, "tri": tri,
            "ones": np.ones((P, P), dtype=ml_dtypes.bfloat16),
        })
    return in_maps


def kernel(x, Wqkv, Wo):
    nc = _get_nc()
    in_maps = _host_inputs(x, Wqkv, Wo)
    res = run_bass_kernel_spmd(nc, in_maps, list(range(N_CORES)))
    y = res.results[0]["y"].astype(np.float64)
    for c in range(1, N_CORES):
        y += res.results[c]["y"].astype(np.float64)
    return y.astype(np.float32).reshape(B, L, H)
